# revision 6
# baseline (speedup 1.0000x reference)
"""CTRNN with per-sample Hebbian plasticity on 8 Trainium2 NeuronCores.

Data-parallel over the sample axis N: each core owns N/8 = 32 samples and
runs the full T-step scan locally; parameters are replicated.

Algorithm (per core). The effective recurrent input is
  rec_t = r_t @ (a*W_rec) + sum_h r_t[n,h] * (a*c*hebb_t)[n,h,k].
The scaled trace A' = sum_{j} gamma_j * r_j (x) r_{j+1} (with the (1-eta)
decay absorbed into gamma_j, "scaled tracking") is kept STALE by up to
W steps in SBUF.  The missing recent rank-1 terms are applied as
attention-style corrections in rows layout: dot products via DVE
tensor_tensor_reduce, per-sample axpy via tensor_scalar with a
per-partition scalar, then a PE transpose-accumulate into the rec PSUM
tile.  Every W steps the window's rank-W update folds into A' with one
K=W bf16 matmul per sample whose operand stacks come from per-sample
strided PE transposes of the tanh-history buffer RT (gamma scaling applied
during the PSUM->SBUF copy via a per-partition scale table).  There are no
DMAs and no departition moves inside the scan.

RT stores tanh(h_t) for every step (f32), so the output projection
tanh(h) @ W_out needs no extra tanh pass; U = a*(x @ W_in + b_rec) is
precomputed before the scan.

Host/dispatch path: the baseline called run_bass_kernel_spmd, whose axon
redirect (run_bass_via_pjrt) rebuilds and re-jits a fresh closure on every
call (~4s/call) and ships f32 x (33.4MB), f32 zero output-init buffers
(33.5MB) and f32 y (33.5MB) over the ~100MB/s axon tunnel.  Here the
jitted shard_map executor is built ONCE and cached; x and y cross the
tunnel as bf16 (half the bytes; x is upcast to f32 on device, y is
produced bf16 by the epilogue); and the output-init operand is a
persistent NON-donated on-device zeros array, so cached calls upload no
output buffer at all.
"""

import numpy as np
from contextlib import ExitStack

import jax
import ml_dtypes
from jax.sharding import Mesh, NamedSharding, PartitionSpec

import concourse.bass as bass
import concourse.tile as tile
from concourse import bacc, bass2jax, mybir, masks

F32 = mybir.dt.float32
BF16 = mybir.dt.bfloat16
FP16 = mybir.dt.float16
AF = mybir.ActivationFunctionType
OP = mybir.AluOpType

T_FULL = 512
N_FULL = 256
I_DIM = 64
H0_DIM = 32
H = 128
O_DIM = 64
N_CORES = 8
NS = N_FULL // N_CORES  # 32 samples per core
G = 4                   # trace groups
GS = NS // G            # 8 samples per group
W = 8                   # fold window (steps)
ABLATE: set = set()    # dev-only: {'mv','corr','fold','rows'} to skip pieces

FP16_NP = np.float16


def build(a: float, e: float, c: float, T: int = T_FULL):
    S = T - 1           # scan steps
    R = S * NS          # rows of X = input_ts[1:] per core
    TR = T * NS         # rows of output per core
    NW = max((S - 1) // W, 1)   # number of folds

    nc = bacc.Bacc("TRN2", target_bir_lowering=False, debug=False)

    x_d = nc.dram_tensor("x", [R, I_DIM], FP16, kind="ExternalInput").ap()
    h0_d = nc.dram_tensor("h0", [NS, H0_DIM], F32, kind="ExternalInput").ap()
    wh0_d = nc.dram_tensor("w_h0", [H0_DIM, H], F32, kind="ExternalInput").ap()
    bh0_d = nc.dram_tensor("b_h0", [H, 1], F32, kind="ExternalInput").ap()
    win_d = nc.dram_tensor("w_in", [I_DIM, H], F32, kind="ExternalInput").ap()
    wrec_d = nc.dram_tensor("w_rec", [H, H], F32, kind="ExternalInput").ap()
    brec_d = nc.dram_tensor("b_rec", [H, 1], F32, kind="ExternalInput").ap()
    wout_d = nc.dram_tensor("w_out", [H, O_DIM], F32, kind="ExternalInput").ap()
    gt_d = nc.dram_tensor("gtab", [128, NW], F32, kind="ExternalInput").ap()
    y_d = nc.dram_tensor("y", [TR, O_DIM], FP16, kind="ExternalOutput").ap()

    with tile.TileContext(nc) as tc, ExitStack() as ctx:
        const = ctx.enter_context(tc.tile_pool(name="const", bufs=1))
        big = ctx.enter_context(tc.tile_pool(name="big", bufs=1))

        ident = const.tile([128, 128], F32)
        masks.make_identity(nc, ident[:])
        w_rec = const.tile([H, H], F32)
        nc.sync.dma_start(w_rec[:], wrec_d)
        w_in = const.tile([I_DIM, H], F32)
        nc.sync.dma_start(w_in[:], win_d)
        w_out = const.tile([H, O_DIM], F32)
        nc.sync.dma_start(w_out[:], wout_d)
        w_h0 = const.tile([H0_DIM, H], F32)
        nc.sync.dma_start(w_h0[:], wh0_d)
        b_h0 = const.tile([H, 1], F32)
        nc.sync.dma_start(b_h0[:], bh0_d)
        b_rec = const.tile([H, 1], F32)
        nc.sync.dma_start(b_rec[:], brec_d)
        gtab = const.tile([128, NW], F32)
        nc.sync.dma_start(gtab[:], gt_d)

        U = big.tile([128, R], F32)        # a*(x@W_in + b_rec), [k, (i, n)]
        RT = big.tile([128, TR], F32)      # tanh(h_t), [k, (t, n)]
        RT3 = RT.rearrange("p (t n) -> p t n", n=NS)
        RT3b = RT.rearrange("p (t n) -> p n t", n=NS)
        A = [big.tile([128, GS * H], BF16, name=f"A{g}", tag=f"A{g}")
             for g in range(G)]            # scaled trace, [h, (n_in_group, k)]
        for g in range(G):
            nc.vector.memset(A[g][:], 0.0)

        # ---- prologue: h0 = h0_data @ W_h0 + b_h0 ----
        hh = ctx.enter_context(tc.tile_pool(name="hh", bufs=2))
        with tc.tile_pool(name="pro", bufs=1) as pro, \
             tc.tile_pool(name="pro_ps", bufs=1, space="PSUM") as pro_ps:
            h0nat = pro.tile([NS, H0_DIM], F32)
            nc.sync.dma_start(h0nat[:], h0_d)
            h0tp = pro_ps.tile([H0_DIM, NS], F32)
            nc.tensor.transpose(h0tp[:], h0nat[:], ident[:NS, :NS])
            h0t = pro.tile([H0_DIM, NS], F32)
            nc.scalar.activation(h0t[:], h0tp[:], AF.Copy)
            h0ps = pro_ps.tile([H, NS], F32)
            nc.tensor.matmul(h0ps[:], lhsT=w_h0[:], rhs=h0t[:], start=True, stop=True)
            h_cur = hh.tile([H, NS], F32, tag="h")
            nc.scalar.activation(h_cur[:], h0ps[:], AF.Identity, bias=b_h0[:, 0:1])

            # ---- prologue: U = a*(X @ W_in + b_rec), transposed ----
            r0 = 0
            while r0 < R:
                rows_n = min(128, R - r0)
                xb = pro.tile([128, I_DIM], FP16, tag="xb", bufs=3)
                nc.sync.dma_start(xb[:rows_n, :], x_d[r0:r0 + rows_n, :])
                xn = pro.tile([128, I_DIM], F32, tag="xn", bufs=3)
                nc.scalar.activation(xn[:rows_n, :], xb[:rows_n, :], AF.Copy)
                xtp = pro_ps.tile([I_DIM, 128], F32, tag="xtp", bufs=2)
                nc.tensor.transpose(xtp[:, :rows_n], xn[:rows_n, :],
                                    ident[:rows_n, :rows_n])
                xt = pro.tile([I_DIM, 128], F32, tag="xt", bufs=3)
                nc.scalar.activation(xt[:, :rows_n], xtp[:, :rows_n], AF.Copy)
                ups = pro_ps.tile([H, 128], F32, tag="ups", bufs=2)
                nc.tensor.matmul(ups[:, :rows_n], lhsT=w_in[:], rhs=xt[:, :rows_n],
                                 start=True, stop=True)
                nc.scalar.activation(U[:, r0:r0 + rows_n], ups[:, :rows_n],
                                     AF.Identity, bias=b_rec[:, 0:1])
                r0 += rows_n

        # ---- main scan ----
        rows = {}
        with tc.tile_pool(name="sm", bufs=2) as sm, \
             tc.tile_pool(name="rr", bufs=W + 2) as rr, \
             tc.tile_pool(name="st", bufs=3) as st, \
             tc.tile_pool(name="ps_rec", bufs=2, space="PSUM") as ps_rec, \
             tc.tile_pool(name="ps_tr", bufs=1, space="PSUM") as ps_tr, \
             tc.tile_pool(name="ps_corr", bufs=1, space="PSUM") as ps_corr, \
             tc.tile_pool(name="ps_fold", bufs=1, space="PSUM") as ps_fold, \
             tc.tile_pool(name="ps_st", bufs=1, space="PSUM") as ps_st:
            for i in range(S):
                beta = (1.0 - e) ** i
                cur = slice(i * NS, (i + 1) * NS)
                slab_i = RT[:, cur]
                nc.scalar.activation(slab_i, h_cur[:], AF.Tanh)       # r_i
                if "rows" in ABLATE:
                    rows[i] = rows.get(i - 1)
                trp = None if "rows" in ABLATE else ps_tr.tile([NS, H], F32, tag="trp")
                if trp is not None:
                    nc.tensor.transpose(trp[:], slab_i, ident[:, :])
                    rows[i] = rr.tile([NS, H], BF16, name="rows", tag="rows")
                    nc.scalar.activation(rows[i][:], trp[:], AF.Copy)

                # fold the last W rank-1 terms into A every W steps.
                # Per 3-sample chunk: two batched transposes build 32-row-
                # aligned stacks (window repeated 4x pads each sample block
                # to 32 partitions), then one K=W bf16 matmul per sample.
                if i % W == 0 and i > 0 and "fold" not in ABLATE:
                    jb, m = i - W, i // W
                    for g in range(G):
                        # contiguous staging of the (sample, window-step)
                        # columns so the stack transposes read unit-stride
                        # weight APs
                        ns0 = g * GS
                        # zero-padded [128, 32]-per-sample staging so every
                        # transpose / matmul uses plain contiguous 32/128
                        # shapes (K=32 with zero rows 8..31)
                        stgL = st.tile([128, GS * 32], F32, tag="stgL")
                        nc.vector.memset(stgL[:], 0.0)
                        stgLv = stgL.rearrange("p (q w) -> p q w", w=32)
                        nc.scalar.activation(
                            stgLv[:, :, 0:W], RT3b[:, ns0:ns0 + GS, jb:i],
                            AF.Copy)
                        stgR = st.tile([128, GS * 32], F32, tag="stgR")
                        nc.vector.memset(stgR[:], 0.0)
                        stgRv = stgR.rearrange("p (q w) -> p q w", w=32)
                        nc.scalar.activation(
                            stgRv[:, :, 0:W], RT3b[:, ns0:ns0 + GS, jb + 1:i + 1],
                            AF.Copy)
                        fps = ps_fold.tile([128, GS * H], F32, tag="fold")
                        for q in range(GS):
                            stpL = ps_st.tile([32, H], F32, tag="stkL")
                            nc.tensor.transpose(stpL[:],
                                                stgL[:, q * 32:(q + 1) * 32],
                                                ident[:, :])
                            lhs_n = st.tile([32, H], BF16, tag="lhs")
                            nc.scalar.activation(lhs_n[:], stpL[:], AF.Copy,
                                                 scale=gtab[0:32, m - 1:m])
                            stpR = ps_st.tile([32, H], F32, tag="stkR")
                            nc.tensor.transpose(stpR[:],
                                                stgR[:, q * 32:(q + 1) * 32],
                                                ident[:, :])
                            rhs_n = st.tile([32, H], BF16, tag="rhs")
                            nc.scalar.activation(rhs_n[:], stpR[:], AF.Copy)
                            nc.tensor.matmul(fps[:, q * H:(q + 1) * H],
                                             lhsT=lhs_n[:], rhs=rhs_n[:],
                                             start=True, stop=True)
                        nc.vector.tensor_tensor(A[g][:], A[g][:], fps[:], OP.add)

                # rec = r @ (a*W_rec) [+ beta * per-sample r^T A] [+ corr]
                # -- one PSUM accumulation group
                B = W * (i // W)
                njs = 0 if "corr" in ABLATE else i - B
                do_mv = i >= W and "mv" not in ABLATE
                rec = ps_rec.tile([H, NS], F32, tag="rec")
                nc.tensor.matmul(rec[:], lhsT=w_rec[:], rhs=slab_i,
                                 start=True, stop=not do_mv)
                if do_mv:
                    rTs = sm.tile([H, NS], BF16, tag="rTs")
                    nc.vector.tensor_scalar(rTs[:], slab_i, beta, None, OP.mult)
                    for n in range(NS):
                        g, j = divmod(n, GS)
                        nc.tensor.matmul(rec[:, n:n + 1],
                                         lhsT=A[g][:, j * H:(j + 1) * H],
                                         rhs=rTs[:, n:n + 1],
                                         start=False,
                                         stop=(n == NS - 1))

                # corrections for unfolded steps j in [B, i) accumulate
                # into their own PSUM tile via transpose-matmuls
                if njs > 0:
                    cps = ps_corr.tile([H, NS], F32, tag="corr")
                    for idx, j in enumerate(range(B, i)):
                        coef = a * c * e * (1.0 - e) ** (i - 1 - j)
                        jk = sm.tile([NS, H], BF16, tag="jk")
                        nc.vector.tensor_tensor(jk[:], rows[i][:], rows[j][:],
                                                OP.mult)
                        dj = sm.tile([NS, 1], F32, tag="dj")
                        nc.vector.tensor_reduce(dj[:], jk[:],
                                                axis=mybir.AxisListType.X,
                                                op=OP.add)
                        tmpj = sm.tile([NS, H], F32, tag="tmpj")
                        nc.vector.tensor_scalar(tmpj[:], rows[j + 1][:],
                                                dj[:, 0:1], coef,
                                                OP.mult, OP.mult)
                        nc.tensor.matmul(cps[:], lhsT=tmpj[:],
                                         rhs=ident[:NS, :NS], is_transpose=True,
                                         start=(idx == 0), stop=(idx == njs - 1))

                # h update
                t3 = sm.tile([H, NS], F32, tag="t3")
                nc.vector.tensor_tensor(t3[:], rec[:], U[:, cur], OP.add)
                if njs > 0:
                    t4 = sm.tile([H, NS], F32, tag="t4")
                    nc.vector.tensor_tensor(t4[:], t3[:], cps[:], OP.add)
                else:
                    t4 = t3
                hsc = sm.tile([H, NS], F32, tag="hsc")
                nc.scalar.activation(hsc[:], h_cur[:], AF.Copy, scale=1.0 - a)
                h_new = hh.tile([H, NS], F32, tag="h")
                nc.vector.tensor_tensor(h_new[:], t4[:], hsc[:], OP.add)
                h_cur = h_new
                rows.pop(i - W - 1, None)

            # final tanh into RT slab S
            nc.scalar.activation(RT[:, S * NS:(S + 1) * NS], h_cur[:], AF.Tanh)

        # ---- epilogue: y = RT @ W_out, back to row-major (bf16) ----
        with tc.tile_pool(name="ep", bufs=3) as ep, \
             tc.tile_pool(name="ep_ps", bufs=2, space="PSUM") as ep_ps:
            r0 = 0
            while r0 < TR:
                rows_n = min(128, TR - r0)
                ops_ = ep_ps.tile([O_DIM, 128], F32, tag="eops")
                nc.tensor.matmul(ops_[:, :rows_n], lhsT=w_out[:],
                                 rhs=RT[:, r0:r0 + rows_n], start=True, stop=True)
                osb = ep.tile([O_DIM, 128], F32, tag="osb")
                nc.scalar.activation(osb[:, :rows_n], ops_[:, :rows_n], AF.Copy)
                otp = ep_ps.tile([128, O_DIM], F32, tag="otp")
                nc.tensor.transpose(otp[:rows_n, :], osb[:, :rows_n],
                                    ident[:O_DIM, :O_DIM])
                ofin = ep.tile([128, O_DIM], FP16, tag="ofin")
                nc.scalar.activation(ofin[:rows_n, :], otp[:rows_n, :], AF.Copy)
                nc.sync.dma_start(y_d[r0:r0 + rows_n, :], ofin[:rows_n, :])
                r0 += rows_n

    nc.compile()
    return nc


def make_gtab(a, e, c, T):
    S = T - 1
    NW = max((S - 1) // W, 1)
    p = np.arange(128) % W
    j = (np.arange(NW)[None, :] * W + p[:, None]).astype(np.float64)
    return (a * c * e * (1.0 - e) ** (-(j + 1.0))).astype(np.float32)


class _Executor:
    """Compile the bass kernel once and keep ONE jitted shard_map callable.

    run_bass_kernel_spmd's axon redirect (run_bass_via_pjrt) rebuilds the
    jit closure per call, so every call re-traces, re-lowers and re-loads
    the NEFF executable (~4s).  It also uploads freshly-allocated zero
    buffers for every ExternalOutput as donated output-init operands.  The
    scan kernel writes every element of y, so the init content is
    irrelevant: we pass a persistent on-device zeros array (not donated,
    created once) instead, and nothing but the real inputs crosses the
    tunnel on a cached call.
    """

    def __init__(self, a, e, c, T):
        self.T = T
        self.nc = nc = build(a, e, c, T)
        bass2jax.install_neuronx_cc_hook()
        assert nc.dbg_addr is None
        partition_name = (nc.partition_id_tensor.name
                          if nc.partition_id_tensor else None)

        in_names, out_names, out_avals = [], [], []
        for alloc in nc.m.functions[0].allocations:
            if not isinstance(alloc, mybir.MemoryLocationSet):
                continue
            name = alloc.memorylocations[0].name
            if alloc.kind == "ExternalInput":
                if name != partition_name:
                    in_names.append(name)
            elif alloc.kind == "ExternalOutput":
                out_names.append(name)
                out_avals.append(jax.core.ShapedArray(
                    tuple(alloc.tensor_shape), mybir.dt.np(alloc.dtype)))
        self.in_names = tuple(in_names)
        self.out_names = tuple(out_names)
        out_avals = tuple(out_avals)
        n_params = len(in_names)
        all_names = tuple(in_names) + tuple(out_names)
        if partition_name is not None:
            all_names = all_names + (partition_name,)

        devices = jax.devices()[:N_CORES]
        assert len(devices) == N_CORES, \
            f"need {N_CORES} devices, have {len(jax.devices())}"
        self.mesh = mesh = Mesh(np.asarray(devices), ("core",))
        self.sharding = NamedSharding(mesh, PartitionSpec("core"))

        def _body(*args):
            operands = list(args)
            if partition_name is not None:
                operands.append(bass2jax.partition_id_tensor())
            return tuple(bass2jax._bass_exec_p.bind(
                *operands, out_avals=out_avals, in_names=all_names,
                out_names=self.out_names, lowering_input_output_aliases=(),
                sim_require_finite=True, sim_require_nnan=True, nc=nc))

        n_out = len(out_names)
        self._fn = jax.jit(jax.shard_map(
            _body, mesh=mesh,
            in_specs=(PartitionSpec("core"),) * (n_params + n_out),
            out_specs=(PartitionSpec("core"),) * n_out,
            check_vma=False), keep_unused=True)

        # persistent (non-donated) output-init operands, resident on device
        self._out_init = [
            jax.device_put(
                np.zeros((N_CORES * av.shape[0], *av.shape[1:]), av.dtype),
                self.sharding)
            for av in out_avals]

    def run(self, cat_in_map):
        args = [cat_in_map[name] for name in self.in_names]
        return self._fn(*args, *self._out_init)


_CACHE: dict = {}


def _get_exec(a, e, c, T) -> _Executor:
    key = (round(a, 9), round(e, 9), round(c, 9), T)
    if key not in _CACHE:
        _CACHE[key] = _Executor(a, e, c, T)
    return _CACHE[key]


def kernel(h0_data, input_ts, W_h0, b_h0, W_in, W_rec, b_rec,
           alpha_rec, W_out, alpha, eta):
    h0_data = np.asarray(h0_data, np.float32)
    input_ts = np.asarray(input_ts, np.float32)
    W_h0 = np.asarray(W_h0, np.float32)
    b_h0 = np.asarray(b_h0, np.float32)
    W_in = np.asarray(W_in, np.float32)
    W_rec = np.asarray(W_rec, np.float32)
    b_rec = np.asarray(b_rec, np.float32)
    alpha_rec = np.asarray(alpha_rec, np.float32)
    W_out = np.asarray(W_out, np.float32)
    a = float(np.asarray(alpha).reshape(-1)[0])
    e = float(np.asarray(eta).reshape(-1)[0])
    c = float(alpha_rec.reshape(-1)[0])
    assert np.allclose(alpha_rec, c), "kernel assumes uniform alpha_rec"

    T, N, _ = input_ts.shape
    ex = _get_exec(a, e, c, T)

    def rep(p):  # replicate a parameter for the 8 shard slots
        return np.broadcast_to(p, (N_CORES,) + p.shape).reshape(
            N_CORES * p.shape[0], *p.shape[1:])

    # x: (T-1, N, I) -> per-core-major (core, t, n, i) rows, bf16 in one pass
    x_cat = np.ascontiguousarray(
        input_ts[1:].reshape(T - 1, N_CORES, NS, I_DIM).transpose(1, 0, 2, 3),
        dtype=FP16_NP).reshape(N_CORES * (T - 1) * NS, I_DIM)
    h0_cat = np.ascontiguousarray(h0_data[0]).reshape(N_CORES * NS, H0_DIM)

    cat = {
        "x": x_cat,
        "h0": h0_cat,
        "w_h0": rep(W_h0),
        "b_h0": rep(b_h0.reshape(H, 1)),
        "w_in": rep(a * W_in),
        "w_rec": rep(a * W_rec),
        "b_rec": rep(a * b_rec.reshape(H, 1)),
        "w_out": rep(W_out),
        "gtab": rep(make_gtab(a, e, c, T)),
    }
    out = ex.run(cat)
    y = np.asarray(out[0])                      # (8*T*NS, O_DIM) bf16
    return np.ascontiguousarray(
        y.reshape(N_CORES, T, NS, O_DIM).transpose(1, 0, 2, 3),
        dtype=np.float32).reshape(T, N, O_DIM)
